# revision 21
# baseline (speedup 1.0000x reference)
"""Directed message-passing GNN (chemprop-style D-MPNN) on 8 Trainium2 cores.

Strategy (node-range sharding, zero collectives, bf16 matmuls):
  - Host sorts edges by target node and splits nodes into 8 contiguous
    ranges of 12500 (edges follow their target's range, ~E/8 per core).
  - The first message-MLP layer is folded on the host into a per-edge
    additive term  ae = Wm1_x.T x[src] + Wm1_e.T ea + bm1  (an input-side
    linear fold, same spirit as fusing Wm2 into W_ih); the node-phase
    input projection  x @ Wn_x + bn  is folded likewise.  All remaining
    compute -- the DEPTH=3 message MLP hidden layer + GRU recurrence, the
    scatter-add aggregation and the 3-layer node MLP -- runs on device in
    bf16 (fp32 PSUM accumulation).
  - Message phase, per 512-edge chunk: all tensors feature-major
    [128, 512]; gate biases ride ScalarE activation bias; additive SBUF
    terms are accumulated into PSUM with identity matmuls to keep the
    DVE/ACT engines off the critical path.
  - Final messages are PE-transposed to edge-major and written to a DRAM
    scratch buffer (8-edge-packed rows) in target-sorted order.
  - Aggregation: per 128-node tile, one indirect-DMA gather of the
    (8-edge-packed) message rows covering its edge span + host-built
    one-hot matrices streamed from DRAM; accumulate  msg.T @ onehot  in
    PSUM, giving feature-major node messages directly.  Node MLP + final
    PE transpose complete the output tile.
"""

import sys

sys.path.insert(0, "/opt/trn_rl_repo")

import numpy as np
from contextlib import ExitStack

import concourse.bass as bass
import concourse.mybir as mybir
import concourse.tile as tile
from concourse.bass import IndirectOffsetOnAxis
from concourse.bass_utils import run_bass_kernel_spmd

# ---------------------------------------------------------------- constants
N_NODES = 100000
N_EDGES = 400000
HIDDEN = 128
NODE_FDIM = 133
EDGE_FDIM = 14
DEPTH = 3
NCORES = 8
P = 128
EC = 512                      # edges per message-phase chunk
NPC = N_NODES // NCORES       # 12500 nodes per core
NT = (NPC + P - 1) // P       # 98 node tiles per core
NPAD = NT * P                 # 12544
F32 = mybir.dt.float32
BF16 = mybir.dt.bfloat16
I32 = mybir.dt.int32
AF = mybir.ActivationFunctionType
ALU = mybir.AluOpType


# ------------------------------------------------ walrus sync-wait limit
def _split_multi_waits(nc):
    """This container's walrus encodes at most ONE sync-wait per
    instruction (any ISA struct). Tile attaches several. Split: insert a
    NoOp per extra wait immediately before the instruction on the same
    engine (sequencer stalls on each in turn)."""
    n_split = 0
    for f in nc.m.functions:
        for bb in f.blocks:
            out = []
            for ins in bb.instructions:
                si = getattr(ins, "sync_info", None)
                waits = list(si.on_wait) if si is not None else []
                if len(waits) > 1:
                    for k, w in enumerate(waits[:-1]):
                        out.append(mybir.InstNoOp(
                            name=f"{ins.name}.w{k}",
                            sync_info=mybir.SyncInfo(on_wait=[w], on_update=[]),
                            bass_nofuse=True,
                            engine=ins.engine,
                        ))
                        n_split += 1
                    ins.sync_info = mybir.SyncInfo(
                        on_wait=[waits[-1]], on_update=list(si.on_update)
                    )
                out.append(ins)
            bb.instructions = out
    return n_split


# ------------------------------------------------------------- host prep
def _prep(inputs):
    """Shard / reorder inputs on the host. Returns (in_maps, meta)."""
    x = np.ascontiguousarray(np.asarray(inputs["x"], np.float32))
    ea = np.ascontiguousarray(np.asarray(inputs["edge_attr"], np.float32))
    ei = np.asarray(inputs["edge_index"])
    src = np.asarray(ei[0], np.int64)
    tgt = np.asarray(ei[1], np.int64)

    f64 = np.float64
    Wm1 = np.asarray(inputs["Wm1"], f64)
    bm1 = np.asarray(inputs["bm1"], f64)
    Wm2 = np.asarray(inputs["Wm2"], f64)
    bm2 = np.asarray(inputs["bm2"], f64)
    W_ih = np.asarray(inputs["W_ih"], f64)
    b_ih = np.asarray(inputs["b_ih"], f64)
    W_hh = np.asarray(inputs["W_hh"], f64)
    b_hh = np.asarray(inputs["b_hh"], f64)
    Wn = np.asarray(inputs["Wn"], f64)
    bn = np.asarray(inputs["bn"], f64)
    Wo1 = np.asarray(inputs["Wo1"], f64)
    bo1 = np.asarray(inputs["bo1"], f64)
    Wo2 = np.asarray(inputs["Wo2"], f64)
    bo2 = np.asarray(inputs["bo2"], f64)

    H = HIDDEN
    # Fuse Wm2 into the GRU input projection: gi = h1 @ (Wm2 @ W_ih.T) + (W_ih@bm2 + b_ih)
    W2G = Wm2 @ W_ih.T                     # [128, 384]
    b2g = W_ih @ bm2 + b_ih                # [384]
    bhh_r, bhh_z, bhh_n = b_hh[:H], b_hh[H:2 * H], b_hh[2 * H:]
    b2g_r, b2g_z, b2g_n = b2g[:H], b2g[H:2 * H], b2g[2 * H:]

    WC = Wm1[147:275]                       # message rows of Wm1
    WHH = W_hh.T                            # [128, 384] gate g at cols gH:(g+1)H
    WNM = Wn[133:261]                       # node-message rows of Wn

    def f32c(a):
        return np.ascontiguousarray(np.asarray(a, np.float32))

    def bf16c(a):
        import ml_dtypes
        return np.ascontiguousarray(
            np.asarray(a, np.float32).astype(ml_dtypes.bfloat16)
        )

    def col(v):
        return f32c(np.asarray(v, f64).reshape(128, 1))

    BHN_LHST = np.zeros((128, 128), f64)
    BHN_LHST[0, :] = bhh_n                  # rank-1 bias via ones-row matmul

    weights = {
        "WC": bf16c(WC), "W2G": bf16c(W2G), "WHH": bf16c(WHH),
        "WNM": bf16c(WNM), "WO1": bf16c(Wo1), "WO2": bf16c(Wo2),
        "BHN_LHST": bf16c(BHN_LHST),
        "IDNB": bf16c(np.eye(128)), "IDNF": f32c(np.eye(128)),
        "BR": col(b2g_r + bhh_r),
        "BZP": col(b2g_z + bhh_z),
        "BZN": col(-(b2g_z + bhh_z)),
        "BGN": col(b2g_n),
        "BHN": col(bhh_n),
        "BO1": col(bo1), "BO2": col(bo2),
    }

    # ---- host folds (input-side linear layers)
    x32 = x.astype(np.float32)
    a_node = x32 @ Wm1[14:147].astype(np.float32)        # [N, 128]
    ae_all = ea @ Wm1[0:14].astype(np.float32)           # [E, 128]
    ae_all += a_node[src]
    ae_all += bm1.astype(np.float32)
    xwn = x32 @ Wn[0:133].astype(np.float32) + bn.astype(np.float32)

    # ---- edge sharding by target-node range
    order = np.argsort(tgt, kind="stable")
    tgt_s = tgt[order]
    bounds = np.searchsorted(tgt_s, NPC * np.arange(NCORES + 1))
    ecounts = np.diff(bounds)
    EPAD = int(np.ceil(ecounts.max() / EC) * EC)
    CH = EPAD // EC
    ROWS8 = EPAD // 8

    import ml_dtypes

    def to_bf16(a32):
        return np.ascontiguousarray(
            np.asarray(a32, np.float32).astype(ml_dtypes.bfloat16)
        )

    # instance count per node tile (uniform across cores) + emission lag:
    # aggregation of tile t may only be emitted after chunk t+LAG's
    # message write (Tile's conservative DRAM-hazard ordering then makes
    # the gather wait for every earlier-emitted write).
    I = 1
    LAG = 0
    per_core = []
    for c in range(NCORES):
        lo, hi = bounds[c], bounds[c + 1]
        tl = tgt_s[lo:hi] - NPC * c
        rp = np.searchsorted(tl, P * np.arange(NT + 1))
        r8_lo = rp[:-1] // 8
        r8_hi = (rp[1:] + 7) // 8
        nrows = np.maximum(r8_hi - r8_lo, 0)
        I = max(I, int(np.ceil(nrows.max() / P)))
        last_chunk = np.maximum(rp[1:] - 1, 0) // EC
        LAG = max(LAG, int((last_chunk - np.arange(NT)).max()))
        per_core.append((lo, hi, tl, rp, r8_lo))
    NI = NT * I
    assert I == 1, f"aggregation assumes one 128-row instance per tile, got {I}"

    in_maps = []
    jj = np.arange(8)
    pp_ = np.arange(P)
    for c in range(NCORES):
        lo, hi, tl, rp, r8_lo = per_core[c]
        ec = hi - lo
        idx = order[lo:hi]

        aeT = np.zeros((128, EPAD), np.float32)
        aeT[:, :ec] = ae_all[idx].T
        aeT = to_bf16(aeT)

        # aggregation gather rows + one-hot scatter matrices
        tlp = np.full(EPAD, 1 << 30, np.int64)
        tlp[:ec] = tl
        aggidx = np.zeros((P, NI), np.int32)
        oh = np.zeros((P, NT, I, 8, P), np.float32)
        for t in range(NT):
            for i in range(I):
                k = t * I + i
                rows = r8_lo[t] + P * i + pp_
                valid = rows * 8 < rp[t + 1]
                rows_c = np.where(valid, rows, 0)
                aggidx[:, k] = rows_c
                e = rows_c[:, None] * 8 + jj[None, :]          # [P, 8]
                seg = tlp[np.minimum(e, EPAD - 1)] - P * t
                ok = (valid[:, None] & (e >= rp[t]) & (e < rp[t + 1])
                      & (seg >= 0) & (seg < P))
                segc = np.where(ok, seg, 0).astype(np.int64)
                block = np.zeros((P, 8, P), np.float32)
                np.put_along_axis(block, segc[:, :, None], 1.0, axis=2)
                block *= ok[:, :, None]
                oh[:, t, i] = block
        aggoh = to_bf16(oh.reshape(P, NI * 8 * P))

        xwnT = np.zeros((128, NPAD), np.float32)
        xwnT[:, :NPC] = xwn[NPC * c:NPC * (c + 1)].T
        xwnT = to_bf16(xwnT)

        m = {
            "aeT": aeT,
            "aggidx": aggidx,
            "aggoh": aggoh,
            "xwnT": xwnT,
        }
        m.update(weights)
        in_maps.append(m)

    meta = {"EPAD": EPAD, "CH": CH, "ROWS8": ROWS8, "I": I, "NI": NI,
            "LAG": LAG}
    return in_maps, meta


# ------------------------------------------------------------ bass program
def _build(meta):
    EPAD, CH, ROWS8, I, NI, LAG = (
        meta["EPAD"], meta["CH"], meta["ROWS8"], meta["I"], meta["NI"],
        meta["LAG"],
    )
    nc = bass.Bass()

    aeT_e = nc.dram_tensor("aeT", [128, EPAD], BF16, kind="ExternalInput")
    aggidx_e = nc.dram_tensor("aggidx", [P, NI], I32, kind="ExternalInput")
    aggoh_e = nc.dram_tensor("aggoh", [P, NI * 8 * P], BF16,
                             kind="ExternalInput")
    xwnT_e = nc.dram_tensor("xwnT", [128, NPAD], BF16, kind="ExternalInput")
    wnames_b = ["WC", "WNM", "WO1", "WO2", "IDNB", "BHN_LHST"]
    w_e = {n: nc.dram_tensor(n, [128, 128], BF16, kind="ExternalInput")
           for n in wnames_b}
    w_e["IDNF"] = nc.dram_tensor("IDNF", [128, 128], F32, kind="ExternalInput")
    w_e["W2G"] = nc.dram_tensor("W2G", [128, 384], BF16, kind="ExternalInput")
    w_e["WHH"] = nc.dram_tensor("WHH", [128, 384], BF16, kind="ExternalInput")
    bnames = ["BR", "BZP", "BZN", "BGN", "BHN", "BO1", "BO2"]
    b_e = {n: nc.dram_tensor(n, [128, 1], F32, kind="ExternalInput")
           for n in bnames}
    out_e = nc.dram_tensor("out", [NPAD, HIDDEN], F32, kind="ExternalOutput")
    msg_e = nc.dram_tensor("msg", [EPAD, HIDDEN], BF16)  # internal scratch

    # 8-edge-packed row view for the aggregation gather
    msg8 = msg_e[:].rearrange("(r s) h -> r (s h)", s=8)
    # message-phase write view: edge = 512c + 128j + 8q + s,
    # SBUF partition p = 8q + s  ->  [c][(q s)=128][j][h]
    msg_w = msg_e[:].rearrange("(c j q s) h -> c (q s) j h", j=4, q=16, s=8)

    with tile.TileContext(nc) as tc, ExitStack() as es:
        cst = es.enter_context(tc.tile_pool(name="cst", bufs=1))
        W = {}
        for n in wnames_b:
            W[n] = cst.tile([128, 128], BF16, tag=n, name=n)
            nc.sync.dma_start(W[n][:], w_e[n][:])
        W["IDNF"] = cst.tile([128, 128], F32, tag="IDNF", name="IDNF")
        nc.sync.dma_start(W["IDNF"][:], w_e["IDNF"][:])
        for n in ("W2G", "WHH"):
            W[n] = cst.tile([128, 384], BF16, tag=n, name=n)
            nc.sync.dma_start(W[n][:], w_e[n][:])
        B = {}
        for n in bnames:
            B[n] = cst.tile([128, 1], F32, tag=n, name=n)
            nc.sync.dma_start(B[n][:], b_e[n][:])
        aggidx = cst.tile([P, NI], I32, tag="aggidx")
        nc.sync.dma_start(aggidx[:], aggidx_e[:])
        ones = cst.tile([128, EC], BF16, tag="ones")
        nc.vector.memset(ones[:], 0.0)
        nc.vector.memset(ones[0:1, :], 1.0)

        ap = es.enter_context(tc.tile_pool(name="ap", bufs=4))
        hp = es.enter_context(tc.tile_pool(name="hp", bufs=6))
        mp = es.enter_context(tc.tile_pool(name="mp", bufs=3))
        gp = es.enter_context(tc.tile_pool(name="gp", bufs=3))
        np_ = es.enter_context(tc.tile_pool(name="np", bufs=3))
        pp = es.enter_context(tc.tile_pool(name="pp", bufs=6, space="PSUM"))
        ppt = es.enter_context(tc.tile_pool(name="ppt", bufs=2, space="PSUM"))

        def psum(n=EC):
            t = pp.tile([128, 512], F32, tag="bank", name="bank")
            return t[:, :n]

        def mm(out, lhsT, rhs, start, stop):
            nc.tensor.matmul(out, lhsT, rhs, start=start, stop=stop)

        IDNB = W["IDNB"]

        # --------------------------------------------- pipelined emission
        # All engine queues are strictly in-order, so cross-engine latency
        # chains (PSUM -> sigmoid -> multiply -> PE ...) are hidden by
        # interleaving the stages of two independent message chunks plus
        # the (LAG-delayed) aggregation tiles.  Stages are split so that
        # an op almost never queues directly behind the op it waits on.
        def chunk_stages(st):
            c = st["c"]
            # S0: load + depth-0 gate matmuls
            ae = ap.tile([128, EC], BF16, tag="ae")
            nc.sync.dma_start(ae[:], aeT_e[:, EC * c:EC * (c + 1)])
            h10 = ap.tile([128, EC], BF16, tag="h10")
            nc.vector.tensor_scalar_max(h10[:], ae[:], 0.0)
            ps_gr = psum()
            ps_gz = psum()
            ps_gn = psum()
            mm(ps_gr, W["W2G"][:, 0:128], h10[:], True, True)
            mm(ps_gz, W["W2G"][:, 128:256], h10[:], True, True)
            mm(ps_gn, W["W2G"][:, 256:384], h10[:], True, False)
            yield
            # S1a: depth-0 sigmoids
            r = hp.tile([128, EC], BF16, tag="r")
            nc.scalar.activation(r[:], ps_gr, AF.Sigmoid, bias=B["BR"][:])
            zb = hp.tile([128, EC], BF16, tag="z")
            nc.scalar.activation(
                zb[:], ps_gz, AF.Sigmoid, bias=B["BZN"][:], scale=-1.0
            )
            tt = hp.tile([128, EC], BF16, tag="tt")
            nc.vector.tensor_scalar_mul(tt[:], r[:], B["BHN"][:])
            ps_m = psum()
            mm(ps_m, IDNB[:], ae[:], True, False)
            yield
            # S1b: depth-0 n/h
            mm(ps_gn, IDNB[:], tt[:], False, True)
            n_t = hp.tile([128, EC], BF16, tag="n")
            nc.scalar.activation(n_t[:], ps_gn, AF.Tanh, bias=B["BGN"][:])
            h = hp.tile([128, EC], BF16, tag="h")
            nc.vector.tensor_mul(h[:], zb[:], n_t[:])
            yield
            for d in range(1, DEPTH):
                # G: hidden-layer + gate matmuls
                mm(ps_m, W["WC"][:], h[:], False, True)
                h1 = hp.tile([128, EC], BF16, tag="h1")
                nc.vector.tensor_scalar_max(h1[:], ps_m, 0.0)
                ps_gr = psum()
                ps_gz = psum()
                ps_gn = psum()
                ps_hn = psum()
                mm(ps_gr, W["W2G"][:, 0:128], h1[:], True, False)
                mm(ps_gr, W["WHH"][:, 0:128], h[:], False, True)
                mm(ps_gz, W["W2G"][:, 128:256], h1[:], True, False)
                mm(ps_gz, W["WHH"][:, 128:256], h[:], False, True)
                mm(ps_gn, W["W2G"][:, 256:384], h1[:], True, False)
                mm(ps_hn, W["WHH"][:, 256:384], h[:], True, False)
                mm(ps_hn, W["BHN_LHST"][:], ones[:], False, True)
                yield
                # Ea: sigmoids + r*(gh_n + bhn)
                r = hp.tile([128, EC], BF16, tag="r")
                nc.scalar.activation(r[:], ps_gr, AF.Sigmoid, bias=B["BR"][:])
                z = hp.tile([128, EC], BF16, tag="z")
                nc.scalar.activation(z[:], ps_gz, AF.Sigmoid, bias=B["BZP"][:])
                p_t = hp.tile([128, EC], BF16, tag="p")
                nc.vector.tensor_mul(p_t[:], r[:], ps_hn)
                if d < DEPTH - 1:
                    ps_m = psum()
                    mm(ps_m, IDNB[:], ae[:], True, False)
                yield
                # Eb: n = tanh(...)
                mm(ps_gn, IDNB[:], p_t[:], False, True)
                n_t = hp.tile([128, EC], BF16, tag="n")
                nc.scalar.activation(n_t[:], ps_gn, AF.Tanh, bias=B["BGN"][:])
                yield
                # Ec: h' = n + z*(h - n)
                s_t = hp.tile([128, EC], BF16, tag="s")
                nc.gpsimd.tensor_sub(s_t[:], h[:], n_t[:])
                u_t = hp.tile([128, EC], BF16, tag="u")
                nc.vector.tensor_mul(u_t[:], z[:], s_t[:])
                h = hp.tile([128, EC], BF16, tag="h")
                nc.vector.tensor_add(h[:], n_t[:], u_t[:])
                yield
            # OUT: transpose to edge-major + DRAM write
            psT = ppt.tile([128, 512], BF16, tag="bankT", name="bankT")
            for j in range(4):
                nc.tensor.transpose(
                    psT[:, P * j:P * (j + 1)], h[:, P * j:P * (j + 1)], IDNB[:]
                )
            mout = mp.tile([128, 4, P], BF16, tag="mout")
            nc.vector.tensor_copy(
                out=mout[:], in_=psT[:].rearrange("p (j h) -> p j h", j=4)
            )
            nc.sync.dma_start(msg_w[c], mout[:])

        def agg_stages(t):
            # A0: gather message rows + stream one-hots / node inputs
            g = gp.tile([P, 8 * P], BF16, tag="mg")
            nc.gpsimd.indirect_dma_start(
                out=g[:],
                out_offset=None,
                in_=msg8,
                in_offset=IndirectOffsetOnAxis(ap=aggidx[:, t:t + 1], axis=0),
            )
            ohl = gp.tile([P, 8 * P], BF16, tag="ohl")
            nc.sync.dma_start(ohl[:], aggoh_e[:, t * 8 * P:(t + 1) * 8 * P])
            xw = np_.tile([P, P], BF16, tag="xw")
            nc.sync.dma_start(xw[:], xwnT_e[:, P * t:P * (t + 1)])
            bank = psum()  # one PSUM bank, sliced per quantity
            yield
            # A1: scatter-add matmuls
            ps_nm = bank[:, 0:P]
            for j in range(8):
                mm(ps_nm, g[:, P * j:P * (j + 1)],
                   ohl[:, P * j:P * (j + 1)], j == 0, j == 7)
            yield
            # A2: node input projection
            nm = np_.tile([P, P], BF16, tag="nm")
            nc.vector.tensor_copy(out=nm[:], in_=ps_nm)
            ps_nr = bank[:, P:2 * P]
            mm(ps_nr, W["WNM"][:], nm[:], True, True)
            nr = np_.tile([P, P], BF16, tag="nr")
            nc.vector.tensor_add(nr[:], xw[:], ps_nr)
            yield
            # A3: output MLP
            ps_o1 = bank[:, 2 * P:3 * P]
            mm(ps_o1, W["WO1"][:], nr[:], True, True)
            s = np_.tile([P, P], BF16, tag="s")
            nc.scalar.activation(s[:], ps_o1, AF.Relu, bias=B["BO1"][:])
            ps_o2 = bank[:, 3 * P:4 * P]
            mm(ps_o2, W["WO2"][:], s[:], True, True)
            yield
            # A4: bias + transpose + write
            oT = np_.tile([P, P], F32, tag="oT")
            nc.vector.tensor_scalar_add(oT[:], ps_o2, B["BO2"][:])
            ps_of = bank[:, 0:P]
            nc.tensor.transpose(ps_of, oT[:], W["IDNF"][:])
            ob = np_.tile([P, P], F32, tag="ob")
            nc.scalar.copy(out=ob[:], in_=ps_of)
            nc.sync.dma_start(out_e[P * t:P * (t + 1), :], ob[:])

        from collections import deque

        chunk_q = deque(range(CH))
        agg_q = deque(range(NT))
        active = []
        done_chunks = [0]

        def refill():
            while (sum(1 for a in active if a[0] == "c") < 2 and chunk_q):
                c = chunk_q.popleft()
                active.append(("c", chunk_stages({"c": c})))
            while (sum(1 for a in active if a[0] == "a") < 2 and agg_q
                   and done_chunks[0] >= min(agg_q[0] + LAG + 1, CH)):
                t = agg_q.popleft()
                active.append(("a", agg_stages(t)))

        refill()
        while active:
            for a in list(active):
                try:
                    next(a[1])
                except StopIteration:
                    active.remove(a)
                    if a[0] == "c":
                        done_chunks[0] += 1
            refill()

    _split_multi_waits(nc)
    return nc


# ---------------------------------------------------------------- kernel
LAST_RESULT = None  # BassKernelResults of the most recent kernel() call


def kernel(**inputs) -> np.ndarray:
    global LAST_RESULT
    in_maps, meta = _prep(inputs)
    nc = _build(meta)
    res = run_bass_kernel_spmd(nc, in_maps, list(range(NCORES)))
    LAST_RESULT = res
    out = np.concatenate(
        [res.results[c]["out"][:NPC] for c in range(NCORES)], axis=0
    )
    return out.astype(np.float32)


if __name__ == "__main__":
    sys.path.insert(0, "/root/problem")
    import reference

    inputs = {k: np.asarray(v) for k, v in reference.setup_inputs().items()}
    exp = np.asarray(reference.reference(**inputs))
    act = kernel(**inputs)
    err = np.abs(act - exp).max() / (np.abs(exp).max() + 1e-12)
    print("Relative error:", err)


# revision 23
# speedup vs baseline: 1.5419x; 1.5419x over previous
"""Directed message-passing GNN (chemprop-style D-MPNN) on 8 Trainium2 cores.

Strategy (node-range sharding, zero collectives, bf16 matmuls):
  - Host sorts edges by target node and splits nodes into 8 contiguous
    ranges of 12500 (edges follow their target's range, ~E/8 per core).
  - The first message-MLP layer is folded on the host into a per-edge
    additive term  ae = Wm1_x.T x[src] + Wm1_e.T ea + bm1  (an input-side
    linear fold, same spirit as fusing Wm2 into W_ih); the node-phase
    input projection  x @ Wn_x + bn  is folded likewise.  All remaining
    compute -- the DEPTH=3 message MLP hidden layer + GRU recurrence, the
    scatter-add aggregation and the 3-layer node MLP -- runs on device in
    bf16 (fp32 PSUM accumulation).
  - Message phase, per 512-edge chunk: all tensors feature-major
    [128, 512]; gate biases ride ScalarE activation bias; additive SBUF
    terms are accumulated into PSUM with identity matmuls to keep the
    DVE/ACT engines off the critical path.
  - Final messages are PE-transposed to edge-major and written to a DRAM
    scratch buffer (8-edge-packed rows) in target-sorted order.
  - Aggregation: per 128-node tile, one indirect-DMA gather of the
    (8-edge-packed) message rows covering its edge span + host-built
    one-hot matrices streamed from DRAM; accumulate  msg.T @ onehot  in
    PSUM, giving feature-major node messages directly.  Node MLP + final
    PE transpose complete the output tile.
"""

import sys

sys.path.insert(0, "/opt/trn_rl_repo")

import numpy as np
from contextlib import ExitStack

import concourse.bass as bass
import concourse.mybir as mybir
import concourse.tile as tile
from concourse.bass import IndirectOffsetOnAxis
from concourse.bass_utils import run_bass_kernel_spmd

# ---------------------------------------------------------------- constants
N_NODES = 100000
N_EDGES = 400000
HIDDEN = 128
NODE_FDIM = 133
EDGE_FDIM = 14
DEPTH = 3
NCORES = 8
P = 128
EC = 512                      # edges per message-phase chunk
NPC = N_NODES // NCORES       # 12500 nodes per core
NT = (NPC + P - 1) // P       # 98 node tiles per core
NPAD = NT * P                 # 12544
F32 = mybir.dt.float32
BF16 = mybir.dt.bfloat16
I32 = mybir.dt.int32
AF = mybir.ActivationFunctionType
ALU = mybir.AluOpType


# ------------------------------------------------ walrus sync-wait limit
def _split_multi_waits(nc):
    """This container's walrus encodes at most ONE sync-wait per
    instruction (any ISA struct). Tile attaches several. Split: insert a
    NoOp per extra wait immediately before the instruction on the same
    engine (sequencer stalls on each in turn)."""
    n_split = 0
    for f in nc.m.functions:
        for bb in f.blocks:
            out = []
            for ins in bb.instructions:
                si = getattr(ins, "sync_info", None)
                waits = list(si.on_wait) if si is not None else []
                if len(waits) > 1:
                    for k, w in enumerate(waits[:-1]):
                        out.append(mybir.InstNoOp(
                            name=f"{ins.name}.w{k}",
                            sync_info=mybir.SyncInfo(on_wait=[w], on_update=[]),
                            bass_nofuse=True,
                            engine=ins.engine,
                        ))
                        n_split += 1
                    ins.sync_info = mybir.SyncInfo(
                        on_wait=[waits[-1]], on_update=list(si.on_update)
                    )
                out.append(ins)
            bb.instructions = out
    return n_split


# ------------------------------------------------------------- host prep
def _prep(inputs):
    """Shard / reorder inputs on the host. Returns (in_maps, meta)."""
    x = np.ascontiguousarray(np.asarray(inputs["x"], np.float32))
    ea = np.ascontiguousarray(np.asarray(inputs["edge_attr"], np.float32))
    ei = np.asarray(inputs["edge_index"])
    src = np.asarray(ei[0], np.int64)
    tgt = np.asarray(ei[1], np.int64)

    f64 = np.float64
    Wm1 = np.asarray(inputs["Wm1"], f64)
    bm1 = np.asarray(inputs["bm1"], f64)
    Wm2 = np.asarray(inputs["Wm2"], f64)
    bm2 = np.asarray(inputs["bm2"], f64)
    W_ih = np.asarray(inputs["W_ih"], f64)
    b_ih = np.asarray(inputs["b_ih"], f64)
    W_hh = np.asarray(inputs["W_hh"], f64)
    b_hh = np.asarray(inputs["b_hh"], f64)
    Wn = np.asarray(inputs["Wn"], f64)
    bn = np.asarray(inputs["bn"], f64)
    Wo1 = np.asarray(inputs["Wo1"], f64)
    bo1 = np.asarray(inputs["bo1"], f64)
    Wo2 = np.asarray(inputs["Wo2"], f64)
    bo2 = np.asarray(inputs["bo2"], f64)

    H = HIDDEN
    # Fuse Wm2 into the GRU input projection: gi = h1 @ (Wm2 @ W_ih.T) + (W_ih@bm2 + b_ih)
    W2G = Wm2 @ W_ih.T                     # [128, 384]
    b2g = W_ih @ bm2 + b_ih                # [384]
    bhh_r, bhh_z, bhh_n = b_hh[:H], b_hh[H:2 * H], b_hh[2 * H:]
    b2g_r, b2g_z, b2g_n = b2g[:H], b2g[H:2 * H], b2g[2 * H:]

    WC = Wm1[147:275]                       # message rows of Wm1
    WHH = W_hh.T                            # [128, 384] gate g at cols gH:(g+1)H
    WNM = Wn[133:261]                       # node-message rows of Wn

    def f32c(a):
        return np.ascontiguousarray(np.asarray(a, np.float32))

    def bf16c(a):
        import ml_dtypes
        return np.ascontiguousarray(
            np.asarray(a, np.float32).astype(ml_dtypes.bfloat16)
        )

    def col(v):
        return f32c(np.asarray(v, f64).reshape(128, 1))

    BHN_LHST = np.zeros((128, 128), f64)
    BHN_LHST[0, :] = bhh_n                  # rank-1 bias via ones-row matmul

    weights = {
        "WC": bf16c(WC), "W2G": bf16c(W2G), "WHH": bf16c(WHH),
        "WNM": bf16c(WNM), "WO1": bf16c(Wo1), "WO2": bf16c(Wo2),
        "BHN_LHST": bf16c(BHN_LHST),
        "IDNB": bf16c(np.eye(128)), "IDNF": f32c(np.eye(128)),
        "BR": col(b2g_r + bhh_r),
        "BZP": col(b2g_z + bhh_z),
        "BZN": col(-(b2g_z + bhh_z)),
        "BGN": col(b2g_n),
        "BHN": col(bhh_n),
        "BO1": col(bo1), "BO2": col(bo2),
    }

    # ---- host folds (input-side linear layers)
    x32 = x.astype(np.float32)
    a_node = x32 @ Wm1[14:147].astype(np.float32)        # [N, 128]
    ae_all = ea @ Wm1[0:14].astype(np.float32)           # [E, 128]
    ae_all += a_node[src]
    ae_all += bm1.astype(np.float32)
    xwn = x32 @ Wn[0:133].astype(np.float32) + bn.astype(np.float32)

    # ---- edge sharding by target-node range
    order = np.argsort(tgt, kind="stable")
    tgt_s = tgt[order]
    bounds = np.searchsorted(tgt_s, NPC * np.arange(NCORES + 1))
    ecounts = np.diff(bounds)
    EPAD = int(np.ceil(ecounts.max() / EC) * EC)
    CH = EPAD // EC
    ROWS8 = EPAD // 8

    import ml_dtypes

    def to_bf16(a32):
        return np.ascontiguousarray(
            np.asarray(a32, np.float32).astype(ml_dtypes.bfloat16)
        )

    # instance count per node tile (uniform across cores) + emission lag:
    # aggregation of tile t may only be emitted after chunk t+LAG's
    # message write (Tile's conservative DRAM-hazard ordering then makes
    # the gather wait for every earlier-emitted write).
    I = 1
    LAG = 0
    per_core = []
    for c in range(NCORES):
        lo, hi = bounds[c], bounds[c + 1]
        tl = tgt_s[lo:hi] - NPC * c
        rp = np.searchsorted(tl, P * np.arange(NT + 1))
        r8_lo = rp[:-1] // 8
        r8_hi = (rp[1:] + 7) // 8
        nrows = np.maximum(r8_hi - r8_lo, 0)
        I = max(I, int(np.ceil(nrows.max() / P)))
        last_chunk = np.maximum(rp[1:] - 1, 0) // EC
        LAG = max(LAG, int((last_chunk - np.arange(NT)).max()))
        per_core.append((lo, hi, tl, rp, r8_lo))
    NI = NT * I
    assert I == 1, f"aggregation assumes one 128-row instance per tile, got {I}"

    in_maps = []
    jj = np.arange(8)
    pp_ = np.arange(P)
    for c in range(NCORES):
        lo, hi, tl, rp, r8_lo = per_core[c]
        ec = hi - lo
        idx = order[lo:hi]

        aeT = np.zeros((128, EPAD), np.float32)
        aeT[:, :ec] = ae_all[idx].T
        aeT = to_bf16(aeT)

        # aggregation gather rows + one-hot scatter matrices
        tlp = np.full(EPAD, 1 << 30, np.int64)
        tlp[:ec] = tl
        aggidx = np.zeros((P, NI), np.int32)
        oh = np.zeros((P, NT, I, 8, P), np.float32)
        for t in range(NT):
            for i in range(I):
                k = t * I + i
                rows = r8_lo[t] + P * i + pp_
                valid = rows * 8 < rp[t + 1]
                rows_c = np.where(valid, rows, 0)
                aggidx[:, k] = rows_c
                e = rows_c[:, None] * 8 + jj[None, :]          # [P, 8]
                seg = tlp[np.minimum(e, EPAD - 1)] - P * t
                ok = (valid[:, None] & (e >= rp[t]) & (e < rp[t + 1])
                      & (seg >= 0) & (seg < P))
                segc = np.where(ok, seg, 0).astype(np.int64)
                block = np.zeros((P, 8, P), np.float32)
                np.put_along_axis(block, segc[:, :, None], 1.0, axis=2)
                block *= ok[:, :, None]
                oh[:, t, i] = block
        aggoh = to_bf16(oh.reshape(P, NI * 8 * P))

        xwnT = np.zeros((128, NPAD), np.float32)
        xwnT[:, :NPC] = xwn[NPC * c:NPC * (c + 1)].T
        xwnT = to_bf16(xwnT)

        m = {
            "aeT": aeT,
            "aggidx": aggidx,
            "aggoh": aggoh,
            "xwnT": xwnT,
        }
        m.update(weights)
        in_maps.append(m)

    meta = {"EPAD": EPAD, "CH": CH, "ROWS8": ROWS8, "I": I, "NI": NI,
            "LAG": LAG}
    return in_maps, meta


# ------------------------------------------------------------ bass program
def _build(meta):
    EPAD, CH, ROWS8, I, NI, LAG = (
        meta["EPAD"], meta["CH"], meta["ROWS8"], meta["I"], meta["NI"],
        meta["LAG"],
    )
    nc = bass.Bass()

    aeT_e = nc.dram_tensor("aeT", [128, EPAD], BF16, kind="ExternalInput")
    aggidx_e = nc.dram_tensor("aggidx", [P, NI], I32, kind="ExternalInput")
    aggoh_e = nc.dram_tensor("aggoh", [P, NI * 8 * P], BF16,
                             kind="ExternalInput")
    xwnT_e = nc.dram_tensor("xwnT", [128, NPAD], BF16, kind="ExternalInput")
    wnames_b = ["WC", "WNM", "WO1", "WO2", "IDNB", "BHN_LHST"]
    w_e = {n: nc.dram_tensor(n, [128, 128], BF16, kind="ExternalInput")
           for n in wnames_b}
    w_e["IDNF"] = nc.dram_tensor("IDNF", [128, 128], F32, kind="ExternalInput")
    w_e["W2G"] = nc.dram_tensor("W2G", [128, 384], BF16, kind="ExternalInput")
    w_e["WHH"] = nc.dram_tensor("WHH", [128, 384], BF16, kind="ExternalInput")
    bnames = ["BR", "BZP", "BZN", "BGN", "BHN", "BO1", "BO2"]
    b_e = {n: nc.dram_tensor(n, [128, 1], F32, kind="ExternalInput")
           for n in bnames}
    out_e = nc.dram_tensor("out", [NPAD, HIDDEN], F32, kind="ExternalOutput")
    msg_e = nc.dram_tensor("msg", [EPAD, HIDDEN], BF16)  # internal scratch

    # 8-edge-packed row view for the aggregation gather
    msg8 = msg_e[:].rearrange("(r s) h -> r (s h)", s=8)
    # message-phase write view: edge = 512c + 128j + 8q + s,
    # SBUF partition p = 8q + s  ->  [c][(q s)=128][j][h]
    msg_w = msg_e[:].rearrange("(c j q s) h -> c (q s) j h", j=4, q=16, s=8)

    with tile.TileContext(nc) as tc, ExitStack() as es:
        cst = es.enter_context(tc.tile_pool(name="cst", bufs=1))
        W = {}
        for n in wnames_b:
            W[n] = cst.tile([128, 128], BF16, tag=n, name=n)
            nc.sync.dma_start(W[n][:], w_e[n][:])
        W["IDNF"] = cst.tile([128, 128], F32, tag="IDNF", name="IDNF")
        nc.sync.dma_start(W["IDNF"][:], w_e["IDNF"][:])
        for n in ("W2G", "WHH"):
            W[n] = cst.tile([128, 384], BF16, tag=n, name=n)
            nc.sync.dma_start(W[n][:], w_e[n][:])
        B = {}
        for n in bnames:
            B[n] = cst.tile([128, 1], F32, tag=n, name=n)
            nc.sync.dma_start(B[n][:], b_e[n][:])
        aggidx = cst.tile([P, NI], I32, tag="aggidx")
        nc.sync.dma_start(aggidx[:], aggidx_e[:])
        ones = cst.tile([128, EC], BF16, tag="ones")
        nc.vector.memset(ones[:], 0.0)
        nc.vector.memset(ones[0:1, :], 1.0)

        ap = es.enter_context(tc.tile_pool(name="ap", bufs=4))
        hp = es.enter_context(tc.tile_pool(name="hp", bufs=6))
        mp = es.enter_context(tc.tile_pool(name="mp", bufs=3))
        gp = es.enter_context(tc.tile_pool(name="gp", bufs=3))
        np_ = es.enter_context(tc.tile_pool(name="np", bufs=3))
        pp = es.enter_context(tc.tile_pool(name="pp", bufs=6, space="PSUM"))
        ppt = es.enter_context(tc.tile_pool(name="ppt", bufs=2, space="PSUM"))

        def psum(n=EC):
            t = pp.tile([128, 512], F32, tag="bank", name="bank")
            return t[:, :n]

        def mm(out, lhsT, rhs, start, stop):
            nc.tensor.matmul(out, lhsT, rhs, start=start, stop=stop)

        IDNB = W["IDNB"]

        # --------------------------------------------- pipelined emission
        # All engine queues are strictly in-order, so cross-engine latency
        # chains (PSUM -> sigmoid -> multiply -> PE ...) are hidden by
        # interleaving the stages of two independent message chunks plus
        # the (LAG-delayed) aggregation tiles.  Stages are split so that
        # an op almost never queues directly behind the op it waits on.
        def chunk_stages(st):
            c = st["c"]
            # S0: load + depth-0 gate matmuls
            ae = ap.tile([128, EC], BF16, tag="ae")
            nc.sync.dma_start(ae[:], aeT_e[:, EC * c:EC * (c + 1)])
            h10 = ap.tile([128, EC], BF16, tag="h10")
            nc.vector.tensor_scalar_max(h10[:], ae[:], 0.0)
            ps_gr = psum()
            ps_gz = psum()
            ps_gn = psum()
            mm(ps_gr, W["W2G"][:, 0:128], h10[:], True, True)
            mm(ps_gz, W["W2G"][:, 128:256], h10[:], True, True)
            mm(ps_gn, W["W2G"][:, 256:384], h10[:], True, False)
            yield
            # S1a: depth-0 sigmoids
            r = hp.tile([128, EC], BF16, tag="r")
            nc.scalar.activation(r[:], ps_gr, AF.Sigmoid, bias=B["BR"][:])
            zb = hp.tile([128, EC], BF16, tag="z")
            nc.scalar.activation(
                zb[:], ps_gz, AF.Sigmoid, bias=B["BZN"][:], scale=-1.0
            )
            tt = hp.tile([128, EC], BF16, tag="tt")
            nc.vector.tensor_scalar_mul(tt[:], r[:], B["BHN"][:])
            ps_m = psum()
            mm(ps_m, IDNB[:], ae[:], True, False)
            yield
            # S1b: depth-0 n/h
            mm(ps_gn, IDNB[:], tt[:], False, True)
            n_t = hp.tile([128, EC], BF16, tag="n")
            nc.scalar.activation(n_t[:], ps_gn, AF.Tanh, bias=B["BGN"][:])
            h = hp.tile([128, EC], BF16, tag="h")
            nc.vector.tensor_mul(h[:], zb[:], n_t[:])
            yield
            for d in range(1, DEPTH):
                # G: hidden-layer + gate matmuls
                mm(ps_m, W["WC"][:], h[:], False, True)
                h1 = hp.tile([128, EC], BF16, tag="h1")
                nc.vector.tensor_scalar_max(h1[:], ps_m, 0.0)
                ps_gr = psum()
                ps_gz = psum()
                ps_gn = psum()
                ps_hn = psum()
                mm(ps_gr, W["W2G"][:, 0:128], h1[:], True, False)
                mm(ps_gr, W["WHH"][:, 0:128], h[:], False, True)
                mm(ps_gz, W["W2G"][:, 128:256], h1[:], True, False)
                mm(ps_gz, W["WHH"][:, 128:256], h[:], False, True)
                mm(ps_gn, W["W2G"][:, 256:384], h1[:], True, False)
                mm(ps_hn, W["WHH"][:, 256:384], h[:], True, False)
                mm(ps_hn, W["BHN_LHST"][:], ones[:], False, True)
                yield
                # Ea: sigmoids + r*(gh_n + bhn)
                r = hp.tile([128, EC], BF16, tag="r")
                nc.scalar.activation(r[:], ps_gr, AF.Sigmoid, bias=B["BR"][:])
                z = hp.tile([128, EC], BF16, tag="z")
                nc.scalar.activation(z[:], ps_gz, AF.Sigmoid, bias=B["BZP"][:])
                p_t = hp.tile([128, EC], BF16, tag="p")
                nc.vector.tensor_mul(p_t[:], r[:], ps_hn)
                if d < DEPTH - 1:
                    ps_m = psum()
                    mm(ps_m, IDNB[:], ae[:], True, False)
                yield
                # Eb: n = tanh(...)
                mm(ps_gn, IDNB[:], p_t[:], False, True)
                n_t = hp.tile([128, EC], BF16, tag="n")
                nc.scalar.activation(n_t[:], ps_gn, AF.Tanh, bias=B["BGN"][:])
                yield
                # Ec: h' = n + z*(h - n)
                s_t = hp.tile([128, EC], BF16, tag="s")
                nc.vector.tensor_sub(s_t[:], h[:], n_t[:])
                u_t = hp.tile([128, EC], BF16, tag="u")
                nc.vector.tensor_mul(u_t[:], z[:], s_t[:])
                h = hp.tile([128, EC], BF16, tag="h")
                nc.vector.tensor_add(h[:], n_t[:], u_t[:])
                yield
            # OUT: transpose to edge-major + DRAM write
            psT = ppt.tile([128, 512], BF16, tag="bankT", name="bankT")
            for j in range(4):
                nc.tensor.transpose(
                    psT[:, P * j:P * (j + 1)], h[:, P * j:P * (j + 1)], IDNB[:]
                )
            mout = mp.tile([128, 4, P], BF16, tag="mout")
            nc.vector.tensor_copy(
                out=mout[:], in_=psT[:].rearrange("p (j h) -> p j h", j=4)
            )
            nc.sync.dma_start(msg_w[c], mout[:])

        def agg_stages(t):
            # A0: gather message rows + stream one-hots / node inputs.
            # The gather's source AP is bounded to the rows chunks <= t+LAG
            # wrote (guaranteed by the host-computed LAG), so the hazard
            # tracker neither waits on later message writes (RAW) nor
            # stalls them behind this read (WAR).
            row_hi = min((t + LAG + 1) * (EC // 8), ROWS8)
            g = gp.tile([P, 8 * P], BF16, tag="mg")
            nc.gpsimd.indirect_dma_start(
                out=g[:],
                out_offset=None,
                in_=msg8[0:row_hi],
                in_offset=IndirectOffsetOnAxis(ap=aggidx[:, t:t + 1], axis=0),
            )
            ohl = gp.tile([P, 8 * P], BF16, tag="ohl")
            nc.sync.dma_start(ohl[:], aggoh_e[:, t * 8 * P:(t + 1) * 8 * P])
            xw = np_.tile([P, P], BF16, tag="xw")
            nc.sync.dma_start(xw[:], xwnT_e[:, P * t:P * (t + 1)])
            bank = psum()  # one PSUM bank, sliced per quantity
            yield
            yield  # padding: let the gather finish before the scatter
            yield  # matmuls reach the head of the Tensor queue
            # A1: scatter-add matmuls
            ps_nm = bank[:, 0:P]
            for j in range(8):
                mm(ps_nm, g[:, P * j:P * (j + 1)],
                   ohl[:, P * j:P * (j + 1)], j == 0, j == 7)
            yield
            # A2: node input projection
            nm = np_.tile([P, P], BF16, tag="nm")
            nc.vector.tensor_copy(out=nm[:], in_=ps_nm)
            ps_nr = bank[:, P:2 * P]
            mm(ps_nr, W["WNM"][:], nm[:], True, True)
            nr = np_.tile([P, P], BF16, tag="nr")
            nc.vector.tensor_add(nr[:], xw[:], ps_nr)
            yield
            # A3: output MLP
            ps_o1 = bank[:, 2 * P:3 * P]
            mm(ps_o1, W["WO1"][:], nr[:], True, True)
            s = np_.tile([P, P], BF16, tag="s")
            nc.scalar.activation(s[:], ps_o1, AF.Relu, bias=B["BO1"][:])
            ps_o2 = bank[:, 3 * P:4 * P]
            mm(ps_o2, W["WO2"][:], s[:], True, True)
            yield
            # A4: bias + transpose + write
            oT = np_.tile([P, P], F32, tag="oT")
            nc.vector.tensor_scalar_add(oT[:], ps_o2, B["BO2"][:])
            ps_of = bank[:, 0:P]
            nc.tensor.transpose(ps_of, oT[:], W["IDNF"][:])
            ob = np_.tile([P, P], F32, tag="ob")
            nc.scalar.copy(out=ob[:], in_=ps_of)
            nc.sync.dma_start(out_e[P * t:P * (t + 1), :], ob[:])

        from collections import deque

        chunk_q = deque(range(CH))
        agg_q = deque(range(NT))
        active = []
        done_chunks = [0]

        def refill():
            while (sum(1 for a in active if a[0] == "c") < 2 and chunk_q):
                c = chunk_q.popleft()
                active.append(("c", chunk_stages({"c": c})))
            while (sum(1 for a in active if a[0] == "a") < 2 and agg_q
                   and done_chunks[0] >= min(agg_q[0] + LAG + 1, CH)):
                t = agg_q.popleft()
                active.append(("a", agg_stages(t)))

        refill()
        while active:
            for a in list(active):
                try:
                    next(a[1])
                except StopIteration:
                    active.remove(a)
                    if a[0] == "c":
                        done_chunks[0] += 1
            refill()

    _split_multi_waits(nc)
    return nc


# ---------------------------------------------------------------- kernel
LAST_RESULT = None  # BassKernelResults of the most recent kernel() call


def kernel(**inputs) -> np.ndarray:
    global LAST_RESULT
    in_maps, meta = _prep(inputs)
    nc = _build(meta)
    res = run_bass_kernel_spmd(nc, in_maps, list(range(NCORES)))
    LAST_RESULT = res
    out = np.concatenate(
        [res.results[c]["out"][:NPC] for c in range(NCORES)], axis=0
    )
    return out.astype(np.float32)


if __name__ == "__main__":
    sys.path.insert(0, "/root/problem")
    import reference

    inputs = {k: np.asarray(v) for k, v in reference.setup_inputs().items()}
    exp = np.asarray(reference.reference(**inputs))
    act = kernel(**inputs)
    err = np.abs(act - exp).max() / (np.abs(exp).max() + 1e-12)
    print("Relative error:", err)
